# revision 8
# baseline (speedup 1.0000x reference)
"""Multi-head self-attention on 8 Trainium2 NeuronCores.

Sharding: core c = b*4 + g handles batch b (of 2) and head-group g (4 heads
of 16). Per core: full qkv projection for its 4 heads, attention, and a
partial output projection (row-slice of Wout). Host sums the 4 partials per
batch and adds bout.

All matmuls run in float32r (fp32 bitcast; FP22 multiply, fp32 accumulate,
1 cycle/row when the moving dim >= 256). Softmax skips max-subtraction
(scaled scores are ~N(0,1); exp overflow impossible in fp32) and gets its
denominator from a ones-column appended to V during the PV matmul.
"""

import os
from contextlib import ExitStack

import numpy as np

import concourse.bass as bass
import concourse.bacc as bacc
import concourse.tile as tile
from concourse import mybir
from concourse._compat import with_exitstack
from concourse.bass_utils import run_bass_kernel_spmd
from concourse.masks import make_identity

B, S, E, H = 2, 2048, 1024, 16
HD = 64
SCALE = HD ** -0.5
NCORES = 8
GROUPS = 4                 # head-groups per batch == cores per batch
HPG = H // GROUPS          # 4 heads per core
DG = HPG * HD              # 256 qkv cols per core per projection
KC = E // 128              # 8 contraction chunks
NT = S // 512              # 4 token chunks of 512
SKT = S // 128             # 16 key tiles of 128
VBLK = 65                  # v block cols: 64 v dims + ones column

FP = mybir.dt.float32
FR = mybir.dt.float32r


@with_exitstack
def _mha_body(ctx: ExitStack, tc: tile.TileContext, xt, w, bqkv, wo, y):
    nc = tc.nc
    main = ctx.enter_context(tc.tile_pool(name="main", bufs=1))

    qT = [main.tile([128, S], FR, name=f"qT{p}") for p in range(2)]
    kT = [main.tile([128, S], FR, name=f"kT{p}") for p in range(2)]
    vT = [main.tile([128, S], FP, name=f"vT{p}") for p in range(2)]
    v_store = main.tile([128, SKT * HPG * VBLK], FR)   # [128, 4160]
    attn = [main.tile([128, S], FR, name=f"attn{p}") for p in range(2)]
    wo_sb = [main.tile([128, E], FR, name=f"wo{p}") for p in range(2)]
    b_sb = main.tile([128, 6], FP)
    identity = main.tile([128, 128], FP)
    make_identity(nc, identity)

    for m in range(6):
        nc.gpsimd.dma_start(out=b_sb[:, m : m + 1], in_=bqkv[m * 128 : (m + 1) * 128, :])

    # ---- phase A: qkv projection (d-major) ----
    with tc.tile_pool(name="xw", bufs=1) as xw:
        xts = [xw.tile([128, S], FR, name=f"xts{k}") for k in range(KC)]
        wts = [xw.tile([128, 768], FR, name=f"wts{k}") for k in range(KC)]
        for k in range(KC):
            nc.default_dma_engine.dma_start(out=xts[k], in_=xt[k * 128 : (k + 1) * 128, :])
            nc.default_dma_engine.dma_start(out=wts[k], in_=w[k * 128 : (k + 1) * 128, :])

        dsts = [qT[0], qT[1], kT[0], kT[1], vT[0], vT[1]]
        with tc.tile_pool(name="qkv_ps", bufs=3, space="PSUM") as qkv_ps:
            for m in range(6):
                for n in range(NT):
                    ps = qkv_ps.tile([128, 512], FP)
                    for k in range(KC):
                        nc.tensor.matmul(
                            ps,
                            wts[k][:, m * 128 : (m + 1) * 128],
                            xts[k][:, n * 512 : (n + 1) * 512],
                            start=(k == 0),
                            stop=(k == KC - 1),
                        )
                    nc.vector.tensor_scalar_add(
                        dsts[m][:, n * 512 : (n + 1) * 512], ps, b_sb[:, m : m + 1]
                    )

        # ---- transpose v into [sk, d] blocks with ones columns ----
        ones_cols = v_store.rearrange("p (j c) -> p j c", c=VBLK)[:, :, 64:65]
        ones_src = xw.tile([128, SKT * HPG], FP)
        nc.vector.memset(ones_src, 1.0)
        nc.vector.tensor_copy(ones_cols, ones_src.rearrange("p (j c) -> p j c", c=1))
        with tc.tile_pool(name="tr_ps", bufs=4, space="PSUM") as tr_ps:
            for t in range(SKT):
                for p in range(2):
                    tp = tr_ps.tile([128, 128], FP)
                    nc.tensor.transpose(
                        tp, vT[p][:, t * 128 : (t + 1) * 128], identity
                    )
                    for hh in range(2):
                        h = 2 * p + hh
                        blk = (t * HPG + h) * VBLK
                        nc.vector.tensor_copy(
                            v_store[:, blk : blk + 64], tp[:, hh * 64 : hh * 64 + 64]
                        )

    # preload Wout during attention
    for p in range(2):
        nc.default_dma_engine.dma_start(out=wo_sb[p], in_=wo[p * 128 : (p + 1) * 128, :])

    # ---- phase B: attention ----
    with tc.tile_pool(name="sc_ps", bufs=3, space="PSUM") as sc_ps, \
         tc.tile_pool(name="pv_ps", bufs=2, space="PSUM") as pv_ps, \
         tc.tile_pool(name="probs", bufs=3) as probs_pool, \
         tc.tile_pool(name="norm", bufs=4) as norm_pool:
        for nq in range(NT):
            for h in range(HPG):
                pi, off = h >> 1, (h & 1) * 64
                att_ps = pv_ps.tile([VBLK, 512], FP)
                for t in range(SKT):
                    s_ps = sc_ps.tile([128, 512], FP)
                    nc.tensor.matmul(
                        s_ps,
                        kT[pi][off : off + 64, t * 128 : (t + 1) * 128],
                        qT[pi][off : off + 64, nq * 512 : (nq + 1) * 512],
                        start=True,
                        stop=True,
                    )
                    pr = probs_pool.tile([128, 512], FR)
                    nc.scalar.activation(
                        pr, s_ps, mybir.ActivationFunctionType.Exp, scale=SCALE
                    )
                    blk = (t * HPG + h) * VBLK
                    nc.tensor.matmul(
                        att_ps,
                        v_store[:, blk : blk + VBLK],
                        pr,
                        start=(t == 0),
                        stop=(t == SKT - 1),
                    )
                rden = norm_pool.tile([1, 512], FP)
                nc.vector.reciprocal(rden, att_ps[64:65, :])
                rden64 = norm_pool.tile([64, 512], FP)
                nc.gpsimd.partition_broadcast(rden64, rden)
                nc.vector.tensor_mul(
                    attn[pi][off : off + 64, nq * 512 : (nq + 1) * 512],
                    att_ps[0:64, :],
                    rden64,
                )

    # ---- phase C: output projection (partial; host sums over groups) ----
    with tc.tile_pool(name="y_ps", bufs=3, space="PSUM") as y_ps, \
         tc.tile_pool(name="y_sb", bufs=3) as y_sb:
        for mt in range(SKT):
            for n2 in range(2):
                ps = y_ps.tile([128, 512], FP)
                for p in range(2):
                    nc.tensor.matmul(
                        ps,
                        attn[p][:, mt * 128 : (mt + 1) * 128],
                        wo_sb[p][:, n2 * 512 : (n2 + 1) * 512],
                        start=(p == 0),
                        stop=(p == 1),
                    )
                yt = y_sb.tile([128, 512], FP)
                nc.vector.tensor_copy(yt, ps)
                nc.default_dma_engine.dma_start(
                    out=y[mt * 128 : (mt + 1) * 128, n2 * 512 : (n2 + 1) * 512], in_=yt
                )


_PROGRAM = None


def _get_program():
    global _PROGRAM
    if _PROGRAM is None:
        nc = bacc.Bacc(
            "TRN2",
            target_bir_lowering=False,
            debug=False,
            enable_asserts=False,
            num_devices=NCORES,
        )
        xt = nc.dram_tensor("xt", [E, S], FR, kind="ExternalInput").ap()
        w = nc.dram_tensor("wqkv", [E, 768], FR, kind="ExternalInput").ap()
        bq = nc.dram_tensor("bqkv", [768, 1], FP, kind="ExternalInput").ap()
        wo = nc.dram_tensor("wout", [DG, E], FR, kind="ExternalInput").ap()
        y = nc.dram_tensor("y", [S, E], FP, kind="ExternalOutput").ap()
        with tile.TileContext(nc) as tc:
            _mha_body(tc, xt, w, bq, wo, y)
        nc.compile()
        _PROGRAM = nc
    return _PROGRAM


LAST_RESULTS = None


def kernel(x, Wqkv, bqkv, Wout, bout):
    global LAST_RESULTS
    x = np.asarray(x, np.float32)
    Wqkv = np.asarray(Wqkv, np.float32)
    bqkv = np.asarray(bqkv, np.float32)
    Wout = np.asarray(Wout, np.float32)
    bout = np.asarray(bout, np.float32)

    nc = _get_program()
    in_maps = []
    for c in range(NCORES):
        b, g = divmod(c, GROUPS)
        # reference layout: Wqkv column j -> head j//192, role (j%192)//64
        idx_q = np.concatenate(
            [np.arange(h * 3 * HD, h * 3 * HD + HD)
             for h in range(g * HPG, (g + 1) * HPG)]
        )
        cols = np.concatenate([idx_q, idx_q + HD, idx_q + 2 * HD])
        w_loc = Wqkv[:, cols]
        b_loc = bqkv[cols][:, None]
        cs = slice(g * DG, (g + 1) * DG)
        in_maps.append({
            "xt": np.ascontiguousarray(x[b].T),
            "wqkv": np.ascontiguousarray(w_loc),
            "bqkv": np.ascontiguousarray(b_loc),
            "wout": np.ascontiguousarray(Wout[cs, :]),
        })

    res = run_bass_kernel_spmd(
        nc,
        in_maps,
        core_ids=list(range(NCORES)),
        trace=bool(int(os.environ.get("KERNEL_TRACE", "0"))),
    )
    LAST_RESULTS = res

    out = np.empty((B, S, E), np.float32)
    for b in range(B):
        acc = res.results[b * GROUPS]["y"].copy()
        for g in range(1, GROUPS):
            acc += res.results[b * GROUPS + g]["y"]
        out[b] = acc + bout[None, :]
    return out


# revision 18
# speedup vs baseline: 1.2483x; 1.2483x over previous
"""Multi-head self-attention on 8 Trainium2 NeuronCores.

Sharding: core c = b*4 + g handles batch b (of 2) and head-group g (4 heads
of 16). Per core: full qkv projection for its 4 heads, attention, and a
partial output projection (row-slice of Wout). Host sums the 4 partials per
batch and adds bout.

x / Wqkv stream in as bf16 (halves input DMA, enables FWL weight loads);
accumulation stays fp32 in PSUM. Attention phase computes scores for two
512-token query blocks back-to-back into a 2-bank PSUM tile so one ACT exp
instruction covers [128,1024], keeping the activation engine (the
bottleneck) streaming while the PE pipelines scores(t+1) ahead of pv(t).
Softmax denominators ride as a ones-column through the PV matmul; their
reciprocals are computed in one fast-approx DVE op per pair.
"""

import os
from contextlib import ExitStack

import ml_dtypes
import numpy as np

import concourse.bass as bass
import concourse.bacc as bacc
import concourse.tile as tile
from concourse import mybir
from concourse._compat import with_exitstack
from concourse.bass_utils import run_bass_kernel_spmd
from concourse.masks import make_identity

B, S, E, H = 2, 2048, 1024, 16
HD = 64
SCALE = HD ** -0.5
NCORES = 8
GROUPS = 4                 # head-groups per batch == cores per batch
HPG = H // GROUPS          # 4 heads per core
DG = HPG * HD              # 256 qkv cols per core per projection
KC = E // 128              # 8 contraction chunks
NT = S // 512              # 4 query chunks of 512
SKT = S // 128             # 16 key tiles of 128
VBLK = 65                  # v block cols: 64 v dims + ones column

FP = mybir.dt.float32
FR = mybir.dt.float32r
BF = mybir.dt.bfloat16


@with_exitstack
def _mha_body(ctx: ExitStack, tc: tile.TileContext, xt, w, bqkv, wo, y):
    nc = tc.nc
    main = ctx.enter_context(tc.tile_pool(name="main", bufs=1))

    qT = [main.tile([128, S], FR, name=f"qT{p}") for p in range(2)]
    kT = [main.tile([128, S], FR, name=f"kT{p}") for p in range(2)]
    vT = [main.tile([128, S], FP, name=f"vT{p}") for p in range(2)]
    v_store = main.tile([128, SKT * HPG * VBLK], FR)   # [128, 4160]
    attn = [main.tile([128, S], FR, name=f"attn{p}") for p in range(2)]
    wo_sb = [main.tile([128, E], FR, name=f"wo{p}") for p in range(2)]
    b_sb = main.tile([128, 6], FP)
    den_all = main.tile([1, 16 * 512], FP)
    rden_all = main.tile([1, 16 * 512], FP)
    identity = main.tile([128, 128], FP)
    make_identity(nc, identity)

    for m in range(6):
        nc.gpsimd.dma_start(out=b_sb[:, m : m + 1], in_=bqkv[m * 128 : (m + 1) * 128, :])

    # ---- phase A: qkv projection (stationary-reuse, 4 psum banks) ----
    with tc.tile_pool(name="xw", bufs=1) as xw:
        xts = [xw.tile([128, S], BF, name=f"xts{k}") for k in range(KC)]
        wts = [xw.tile([128, 768], BF, name=f"wts{k}") for k in range(KC)]
        for k in range(KC):
            nc.default_dma_engine.dma_start(out=xts[k], in_=xt[k * 128 : (k + 1) * 128, :])
            nc.default_dma_engine.dma_start(out=wts[k], in_=w[k * 128 : (k + 1) * 128, :])

        dsts = [qT[0], qT[1], kT[0], kT[1], vT[0], vT[1]]
        with tc.tile_pool(name="qkv_ps", bufs=2, space="PSUM") as qkv_ps:
            for m in range(6):
                pss = [qkv_ps.tile([128, 512], FP, name=f"qps{n}") for n in range(NT)]
                for k in range(KC):
                    for n in range(NT):
                        nc.tensor.matmul(
                            pss[n],
                            wts[k][:, m * 128 : (m + 1) * 128],
                            xts[k][:, n * 512 : (n + 1) * 512],
                            start=(k == 0),
                            stop=(k == KC - 1),
                        )
                for n in range(NT):
                    nc.vector.tensor_scalar_add(
                        dsts[m][:, n * 512 : (n + 1) * 512], pss[n], b_sb[:, m : m + 1]
                    )

        # ---- transpose v into [sk, d] blocks with ones columns ----
        ones_cols = v_store.rearrange("p (j c) -> p j c", c=VBLK)[:, :, 64:65]
        ones_src = xw.tile([128, SKT * HPG], FP)
        nc.vector.memset(ones_src, 1.0)
        nc.vector.tensor_copy(ones_cols, ones_src.rearrange("p (j c) -> p j c", c=1))
        with tc.tile_pool(name="tr_ps", bufs=4, space="PSUM") as tr_ps:
            for t in range(SKT):
                for p in range(2):
                    tp = tr_ps.tile([128, 128], FP)
                    nc.tensor.transpose(
                        tp, vT[p][:, t * 128 : (t + 1) * 128], identity
                    )
                    for hh in range(2):
                        h = 2 * p + hh
                        blk = (t * HPG + h) * VBLK
                        nc.vector.tensor_copy(
                            v_store[:, blk : blk + 64], tp[:, hh * 64 : hh * 64 + 64]
                        )

    # preload Wout during attention
    for p in range(2):
        nc.default_dma_engine.dma_start(out=wo_sb[p], in_=wo[p * 128 : (p + 1) * 128, :])

    # ---- phase B: attention ----
    with tc.tile_pool(name="sc_ps", bufs=2, space="PSUM") as sc_ps, \
         tc.tile_pool(name="pv_ps", bufs=2, space="PSUM") as pv_ps, \
         tc.tile_pool(name="probs", bufs=5) as probs_pool, \
         tc.tile_pool(name="bcast", bufs=3) as bcast_pool:
        for h in range(HPG):
            pi, off = h >> 1, (h & 1) * 64
            for npair in range(2):
                atts = [
                    pv_ps.tile([VBLK, 512], FP, name=f"att{j}")
                    for j in range(2)
                ]
                for t in range(SKT):
                    s2 = sc_ps.tile([128, 1024], FP, name="s2")
                    for j in range(2):
                        nq = npair * 2 + j
                        nc.tensor.matmul(
                            s2[:, j * 512 : (j + 1) * 512],
                            kT[pi][off : off + 64, t * 128 : (t + 1) * 128],
                            qT[pi][off : off + 64, nq * 512 : (nq + 1) * 512],
                            start=True,
                            stop=True,
                        )
                    pr2 = probs_pool.tile([128, 1024], FR, name="pr2")
                    nc.scalar.activation(
                        pr2, s2, mybir.ActivationFunctionType.Exp, scale=SCALE
                    )
                    blk = (t * HPG + h) * VBLK
                    for j in range(2):
                        nc.tensor.matmul(
                            atts[j],
                            v_store[:, blk : blk + VBLK],
                            pr2[:, j * 512 : (j + 1) * 512],
                            start=(t == 0),
                            stop=(t == SKT - 1),
                        )
                # drain PSUM: unnormalized attn rows + denominators
                r0 = h * NT + npair * 2
                for j in range(2):
                    nq = npair * 2 + j
                    nc.vector.tensor_copy(
                        attn[pi][off : off + 64, nq * 512 : (nq + 1) * 512],
                        atts[j][0:64, :],
                    )
                    nc.vector.tensor_copy(
                        den_all[:, (r0 + j) * 512 : (r0 + j + 1) * 512],
                        atts[j][64:65, :],
                    )
                nc.vector.reciprocal_approx_fast(
                    rden_all[:, r0 * 512 : (r0 + 2) * 512],
                    den_all[:, r0 * 512 : (r0 + 2) * 512],
                )
                for j in range(2):
                    nq = npair * 2 + j
                    rden128 = bcast_pool.tile([128, 512], FP, name="rb")
                    nc.gpsimd.partition_broadcast(
                        rden128, rden_all[:, (r0 + j) * 512 : (r0 + j + 1) * 512]
                    )
                    sl = attn[pi][off : off + 64, nq * 512 : (nq + 1) * 512]
                    nc.vector.tensor_mul(sl, sl, rden128[off : off + 64, :])

    # ---- phase C: output projection (partial; host sums over groups) ----
    with tc.tile_pool(name="y_ps", bufs=4, space="PSUM") as y_ps, \
         tc.tile_pool(name="y_sb", bufs=4) as y_sb:
        for mt in range(SKT):
            for n2 in range(2):
                ps = y_ps.tile([128, 512], FP)
                for p in range(2):
                    nc.tensor.matmul(
                        ps,
                        attn[p][:, mt * 128 : (mt + 1) * 128],
                        wo_sb[p][:, n2 * 512 : (n2 + 1) * 512],
                        start=(p == 0),
                        stop=(p == 1),
                    )
                yt = y_sb.tile([128, 512], BF)
                if n2 == 0:
                    nc.vector.tensor_copy(yt, ps)
                else:
                    nc.scalar.copy(yt, ps)
                nc.default_dma_engine.dma_start(
                    out=y[mt * 128 : (mt + 1) * 128, n2 * 512 : (n2 + 1) * 512], in_=yt
                )


_PROGRAM = None


def _get_program():
    global _PROGRAM
    if _PROGRAM is None:
        nc = bacc.Bacc(
            "TRN2",
            target_bir_lowering=False,
            debug=False,
            enable_asserts=False,
            num_devices=NCORES,
        )
        xt = nc.dram_tensor("xt", [E, S], BF, kind="ExternalInput").ap()
        w = nc.dram_tensor("wqkv", [E, 768], BF, kind="ExternalInput").ap()
        bq = nc.dram_tensor("bqkv", [768, 1], FP, kind="ExternalInput").ap()
        wo = nc.dram_tensor("wout", [DG, E], FR, kind="ExternalInput").ap()
        y = nc.dram_tensor("y", [S, E], BF, kind="ExternalOutput").ap()
        with tile.TileContext(nc) as tc:
            _mha_body(tc, xt, w, bq, wo, y)
        nc.compile()
        _PROGRAM = nc
    return _PROGRAM


LAST_RESULTS = None


def kernel(x, Wqkv, bqkv, Wout, bout):
    global LAST_RESULTS
    x = np.asarray(x, np.float32)
    Wqkv = np.asarray(Wqkv, np.float32)
    bqkv = np.asarray(bqkv, np.float32)
    Wout = np.asarray(Wout, np.float32)
    bout = np.asarray(bout, np.float32)

    nc = _get_program()
    in_maps = []
    for c in range(NCORES):
        b, g = divmod(c, GROUPS)
        # reference layout: Wqkv column j -> head j//192, role (j%192)//64
        idx_q = np.concatenate(
            [np.arange(h * 3 * HD, h * 3 * HD + HD)
             for h in range(g * HPG, (g + 1) * HPG)]
        )
        cols = np.concatenate([idx_q, idx_q + HD, idx_q + 2 * HD])
        w_loc = Wqkv[:, cols]
        b_loc = bqkv[cols][:, None]
        cs = slice(g * DG, (g + 1) * DG)
        in_maps.append({
            "xt": np.ascontiguousarray(x[b].T).astype(ml_dtypes.bfloat16),
            "wqkv": np.ascontiguousarray(w_loc).astype(ml_dtypes.bfloat16),
            "bqkv": np.ascontiguousarray(b_loc),
            "wout": np.ascontiguousarray(Wout[cs, :]),
        })

    res = run_bass_kernel_spmd(
        nc,
        in_maps,
        core_ids=list(range(NCORES)),
        trace=bool(int(os.environ.get("KERNEL_TRACE", "0"))),
    )
    LAST_RESULTS = res

    out = np.empty((B, S, E), np.float32)
    for b in range(B):
        acc = res.results[b * GROUPS]["y"].astype(np.float32)
        for g in range(1, GROUPS):
            acc += res.results[b * GROUPS + g]["y"].astype(np.float32)
        out[b] = acc + bout[None, :]
    return out
